# revision 1
# baseline (speedup 1.0000x reference)
"""Trainium2 Bass kernel for nn_CGRegressorAdapter (GNN message passing).

Strategy:
  - Data-parallel over B=32 graphs: 8 cores x 4 graphs each. Weights replicated.
  - Per-graph dense adjacency AT[src, dst] (edge-count matrix) built on host
    from edge_index (pure integer layout prep), shipped bf16 (counts are exact).
  - All node states kept transposed [128 feat, 2048 nodes] in f32.
  - GraphConv: m = h @ Wnbr via f32 PE matmuls; m split into bf16 hi+lo;
    agg^T accumulated as (m_hi^T + m_lo^T) @ AT rows streamed 512-wide (bf16 PE),
    plus f32 Wself path, all into the same PSUM; fused bias+ReLU on ACT.
  - Last-node extraction via one-hot column mask + DVE multiply-reduce.
  - Small regression head entirely on-chip in f32.
"""
import numpy as np
import ml_dtypes

import concourse.bass as bass
import concourse.mybir as mybir
from concourse import bacc
from concourse.bass import ts
from concourse.bass_utils import run_bass_kernel_spmd
from concourse.tile import TileContext

BF16 = ml_dtypes.bfloat16
F32 = np.float32

B, N, E, H, L, VOCAB = 32, 2048, 8192, 128, 4, 32
N_CORES = 8
NG = B // N_CORES          # graphs per core
NJ = N // 128              # 16 src chunks
NSPAN = N // 512           # 4 psum spans
dt = mybir.dt
Alu = mybir.AluOpType
Act = mybir.ActivationFunctionType

# bias column indices in the packed bias tile
BCOL_BASE = 0      # 0..3  base_b
BCOL_ADAPT = 4     # 4..7  adapt_b
BCOL_HB1 = 8
BCOL_HMID = 9      # 9..11
BCOL_HB5 = 12
NBCOL = 16


def _build_program(n_graphs=NG, l_base=L, l_adapt=L, do_head=True):
    nc = bacc.Bacc("TRN2", target_bir_lowering=False, debug=False,
                   num_devices=N_CORES)
    f32, bf16 = dt.float32, dt.bfloat16

    at_d = nc.declare_dram_parameter("at", [NG * NJ, 128, N], bf16, isOutput=False)
    erhs_d = nc.declare_dram_parameter("embed_rhs", [NG, 128, N], f32, isOutput=False)
    sel_d = nc.declare_dram_parameter("selrep", [NG, 128, N], bf16, isOutput=False)
    embw_d = nc.declare_dram_parameter("embed_w", [128, H], f32, isOutput=False)
    bws_d = nc.declare_dram_parameter("bwself", [L, H, H], f32, isOutput=False)
    bwn_d = nc.declare_dram_parameter("bwnbr", [L, H, H], f32, isOutput=False)
    aws_d = nc.declare_dram_parameter("awself", [L, H, 2, H], f32, isOutput=False)
    awn_d = nc.declare_dram_parameter("awnbr", [L, H, 2, H], f32, isOutput=False)
    hw1_d = nc.declare_dram_parameter("hw1", [H, 2, H], f32, isOutput=False)
    hwm_d = nc.declare_dram_parameter("hwmid", [H, 3, H], f32, isOutput=False)
    hw5_d = nc.declare_dram_parameter("hw5", [H, 1], f32, isOutput=False)
    bias_d = nc.declare_dram_parameter("biases", [H, NBCOL], f32, isOutput=False)
    y_d = nc.declare_dram_parameter("y", [1, NG], f32, isOutput=True)

    with TileContext(nc) as tc:
        with (
            tc.tile_pool(name="const", bufs=1) as const,
            tc.tile_pool(name="atp", bufs=1) as atp,
            tc.tile_pool(name="state", bufs=1) as state,
            tc.tile_pool(name="currp", bufs=2) as currp,
            tc.tile_pool(name="mp", bufs=4) as mp,
            tc.tile_pool(name="work", bufs=2) as work,
            tc.tile_pool(name="psum_agg", bufs=1, space="PSUM") as psum_agg,
            tc.tile_pool(name="psum_m", bufs=4, space="PSUM") as psum_m,
        ):
            # ---- constants ----
            embw = const.tile([128, H], f32)
            nc.sync.dma_start(embw[:], embw_d[:])
            bias_t = const.tile([H, NBCOL], f32)
            nc.sync.dma_start(bias_t[:], bias_d[:])
            bws_t = []
            bwn_t = []
            aws_t = []
            awn_t = []
            for i in range(L):
                w = const.tile([H, H], f32, tag=f"bws{i}")
                nc.sync.dma_start(w[:], bws_d[i])
                bws_t.append(w)
                w = const.tile([H, H], f32, tag=f"bwn{i}")
                nc.sync.dma_start(w[:], bwn_d[i])
                bwn_t.append(w)
                w = const.tile([H, 2, H], f32, tag=f"aws{i}")
                nc.sync.dma_start(w[:], aws_d[i])
                aws_t.append(w)
                w = const.tile([H, 2, H], f32, tag=f"awn{i}")
                nc.sync.dma_start(w[:], awn_d[i])
                awn_t.append(w)
            hw1_t = const.tile([H, 2, H], f32)
            nc.sync.dma_start(hw1_t[:], hw1_d[:])
            hwm_t = const.tile([H, 3, H], f32)
            nc.sync.dma_start(hwm_t[:], hwm_d[:])
            hw5_t = const.tile([H, 1], f32)
            nc.sync.dma_start(hw5_t[:], hw5_d[:])
            dummy = const.tile([128, 1], f32)

            gbT = state.tile([128, NG], f32, tag="gb")
            gaT = state.tile([128, NG], f32, tag="ga")

            def gconv(nbr_srcs, self_srcs, at_tiles, bias_col, out_tile,
                      relu=True):
                """nbr_srcs: list of (stateT_tile, Wnbr_rhs_ap [128,H]).
                self_srcs: list of (stateT_tile, Wself_lhsT_ap [128,H]).
                out_tile: [128, N] f32 output state."""
                aggs = [psum_agg.tile([128, 512], f32, tag=f"agg{s}",
                                      name=f"agg{s}") for s in range(NSPAN)]

                def emit_m(j):
                    pm = psum_m.tile([128, 128], f32, tag="pm")
                    nlast = len(nbr_srcs) - 1
                    for idx, (src, w) in enumerate(nbr_srcs):
                        nc.tensor.matmul(pm[:], src[:, ts(j, 128)], w,
                                         start=(idx == 0), stop=(idx == nlast))
                    mhi = mp.tile([128, 128], bf16, tag="mhi")
                    nc.scalar.copy(mhi[:], pm[:])
                    mlo = mp.tile([128, 128], bf16, tag="mlo")
                    nc.vector.tensor_sub(out=mlo[:], in0=pm[:], in1=mhi[:])
                    return mhi, mlo

                # first m chunks before the self path so the hi/lo casts hide
                # under the self matmuls and the A-phase starts stall-free
                m_next = emit_m(0)
                # self path: f32, wide rhs
                for idx, (src, w) in enumerate(self_srcs):
                    for s in range(NSPAN):
                        nc.tensor.matmul(aggs[s][:], w, src[:, ts(s, 512)],
                                         start=(idx == 0), stop=False)
                for j in range(NJ):
                    mhi, mlo = m_next
                    if j + 1 < NJ:
                        m_next = emit_m(j + 1)
                    for s in range(NSPAN):
                        nc.tensor.matmul(aggs[s][:], mhi[:],
                                         at_tiles[j][:, ts(s, 512)],
                                         start=False, stop=False)
                    for s in range(NSPAN):
                        nc.tensor.matmul(aggs[s][:], mlo[:],
                                         at_tiles[j][:, ts(s, 512)],
                                         start=False, stop=(j == NJ - 1))
                func = Act.Relu if relu else Act.Identity
                for s in range(NSPAN):
                    nc.scalar.activation(out_tile[:, ts(s, 512)],
                                         aggs[s][:], func,
                                         bias=bias_t[:, bias_col:bias_col + 1])

            for g in range(n_graphs):
                at_t = []
                for j in range(NJ):
                    t = atp.tile([128, N], bf16, tag=f"at{j}", name=f"at{j}")
                    nc.sync.dma_start(t[:], at_d[g * NJ + j])
                    at_t.append(t)
                # ---- embed ----
                erhs = work.tile([128, N], f32, tag="erhs")
                nc.sync.dma_start(erhs[:], erhs_d[g])
                selt = work.tile([128, N], bf16, tag="sel")
                nc.sync.dma_start(selt[:], sel_d[g])

                lat = [state.tile([128, N], f32, tag=f"lat{k}", name=f"lat{k}")
                       for k in range(L + 1)]
                eaggs = [psum_agg.tile([128, 512], f32, tag=f"agg{s}",
                                       name=f"eagg{s}") for s in range(NSPAN)]
                for s in range(NSPAN):
                    nc.tensor.matmul(eaggs[s][:], embw[:],
                                     erhs[:, ts(s, 512)], start=True, stop=True)
                for s in range(NSPAN):
                    nc.scalar.copy(lat[0][:, ts(s, 512)], eaggs[s][:])

                # ---- base stack ----
                for i in range(l_base):
                    gconv(nbr_srcs=[(lat[i], bwn_t[i][:])],
                          self_srcs=[(lat[i], bws_t[i][:])],
                          at_tiles=at_t, bias_col=BCOL_BASE + i,
                          out_tile=lat[i + 1])

                # ---- adapter stack ----
                curr = lat[0]
                for i in range(l_adapt):
                    ncurr = currp.tile([128, N], f32, tag="curr")
                    gconv(nbr_srcs=[(lat[i + 1], awn_t[i][:, 0, :]),
                                    (curr, awn_t[i][:, 1, :])],
                          self_srcs=[(lat[i + 1], aws_t[i][:, 0, :]),
                                     (curr, aws_t[i][:, 1, :])],
                          at_tiles=at_t, bias_col=BCOL_ADAPT + i,
                          out_tile=ncurr)
                    curr = ncurr

                # ---- last-node extraction (mask-multiply + reduce) ----
                extr = work.tile([128, N], f32, tag="extr")
                nc.vector.tensor_mul(out=extr[:], in0=lat[l_base][:], in1=selt[:])
                nc.vector.tensor_reduce(gbT[:, g:g + 1], extr[:],
                                        mybir.AxisListType.X, Alu.add)
                extr2 = work.tile([128, N], f32, tag="extr")
                nc.vector.tensor_mul(out=extr2[:], in0=curr[:], in1=selt[:])
                nc.vector.tensor_reduce(gaT[:, g:g + 1], extr2[:],
                                        mybir.AxisListType.X, Alu.add)

            if do_head:
                # ---- regression head (all graphs at once) ----
                def head_mm(lhsT, rhs, bias_col, func):
                    pm = psum_m.tile([128, 128], f32, tag="pm")
                    nc.tensor.matmul(pm[:, :NG], lhsT, rhs, start=True, stop=True)
                    out = state.tile([128, NG], f32, tag="hy")
                    nc.scalar.activation(out[:], pm[:, :NG], func,
                                         bias=bias_t[:, bias_col:bias_col + 1])
                    return out

                pm = psum_m.tile([128, 128], f32, tag="pm")
                nc.tensor.matmul(pm[:, :NG], hw1_t[:, 0, :], gbT[:], start=True, stop=False)
                nc.tensor.matmul(pm[:, :NG], hw1_t[:, 1, :], gaT[:], start=False, stop=True)
                y1 = state.tile([128, NG], f32, tag="hy")
                nc.scalar.activation(y1[:], pm[:, :NG], Act.Identity,
                                     bias=bias_t[:, BCOL_HB1:BCOL_HB1 + 1])
                y2 = head_mm(hwm_t[:, 0, :], y1[:], BCOL_HMID + 0, Act.Relu)
                y3 = head_mm(hwm_t[:, 1, :], y2[:], BCOL_HMID + 1, Act.Identity)
                y4 = head_mm(hwm_t[:, 2, :], y3[:], BCOL_HMID + 2, Act.Relu)
                pm5 = psum_m.tile([128, 128], f32, tag="pm")
                nc.tensor.matmul(pm5[:1, :NG], hw5_t[:], y4[:], start=True, stop=True)
                yout = state.tile([1, NG], f32, tag="yout")
                nc.scalar.activation(yout[:], pm5[:1, :NG], Act.Identity,
                                     bias=bias_t[:1, BCOL_HB5:BCOL_HB5 + 1])
                nc.sync.dma_start(y_d[:], yout[:])
            else:
                yout = state.tile([1, NG], f32, tag="yout")
                nc.vector.tensor_copy(out=yout[:], in_=gbT[:1, :])
                nc.sync.dma_start(y_d[:], yout[:])

    nc.compile()
    return nc


_NC_CACHE = {}


def _get_program():
    if "nc" not in _NC_CACHE:
        _NC_CACHE["nc"] = _build_program()
    return _NC_CACHE["nc"]


def _prep_inputs(inputs):
    """Host-side sharding + layout prep. Returns list of per-core in_maps."""
    inds = np.asarray(inputs["regular_node_inds"]).astype(np.int64)
    shapes = np.asarray(inputs["regular_node_shapes"], dtype=F32)
    edge = np.asarray(inputs["edge_index"]).astype(np.int64)
    last_idx = np.asarray(inputs["last_idx"]).astype(np.int64)

    # adjacency AT[src, dst] counts per graph, bf16 (exact small ints)
    at_all = np.zeros((B, N, N), dtype=F32)
    for g in range(B):
        np.add.at(at_all[g], (edge[g, 0], edge[g, 1]), 1.0)
    at_all = at_all.astype(BF16)

    # embed rhs: rows 0..31 one-hot(inds)^T, rows 32..35 shapes^T, rest 0
    erhs_all = np.zeros((B, 128, N), dtype=F32)
    ar = np.arange(N)
    for g in range(B):
        erhs_all[g, inds[g], ar] = 1.0
        erhs_all[g, VOCAB:VOCAB + 4, :] = shapes[g].T
    # last-node selection mask replicated over partitions
    sel_all = np.zeros((B, 128, N), dtype=BF16)
    for g in range(B):
        sel_all[g, :, last_idx[g]] = 1.0

    embed_w = np.zeros((128, H), dtype=F32)
    embed_w[:VOCAB] = np.asarray(inputs["embed_table"], dtype=F32)
    embed_w[VOCAB:VOCAB + 4] = np.asarray(inputs["shape_w"], dtype=F32)

    aws = np.asarray(inputs["adapt_Wself"], dtype=F32).reshape(L, 2, H, H)
    awn = np.asarray(inputs["adapt_Wnbr"], dtype=F32).reshape(L, 2, H, H)
    aws = np.ascontiguousarray(aws.transpose(0, 2, 1, 3))  # [L, H, 2, H]
    awn = np.ascontiguousarray(awn.transpose(0, 2, 1, 3))
    hw1 = np.ascontiguousarray(
        np.asarray(inputs["hW1"], dtype=F32).reshape(2, H, H).transpose(1, 0, 2))

    biases = np.zeros((H, NBCOL), dtype=F32)
    biases[:, BCOL_BASE:BCOL_BASE + L] = np.asarray(inputs["base_b"], dtype=F32).T
    biases[:, BCOL_ADAPT:BCOL_ADAPT + L] = np.asarray(inputs["adapt_b"], dtype=F32).T
    biases[:, BCOL_HB1] = np.asarray(inputs["hb1"], dtype=F32)
    biases[:, BCOL_HMID:BCOL_HMID + 3] = np.asarray(inputs["hbmid"], dtype=F32).T
    biases[0, BCOL_HB5] = np.asarray(inputs["hb5"], dtype=F32)[0]

    shared = {
        "embed_w": embed_w,
        "bwself": np.asarray(inputs["base_Wself"], dtype=F32),
        "bwnbr": np.asarray(inputs["base_Wnbr"], dtype=F32),
        "awself": aws,
        "awnbr": awn,
        "hw1": hw1,
        "hwmid": np.ascontiguousarray(
            np.asarray(inputs["hWmid"], dtype=F32).transpose(1, 0, 2)),
        "hw5": np.asarray(inputs["hW5"], dtype=F32),
        "biases": biases,
    }
    in_maps = []
    for c in range(N_CORES):
        g0 = c * NG
        in_maps.append({
            "at": np.ascontiguousarray(
                at_all[g0:g0 + NG].reshape(NG * NJ, 128, N)),
            "embed_rhs": erhs_all[g0:g0 + NG],
            "selrep": sel_all[g0:g0 + NG],
            **shared,
        })
    return in_maps


def kernel(**inputs) -> np.ndarray:
    nc = _get_program()
    in_maps = _prep_inputs(inputs)
    res = run_bass_kernel_spmd(nc, in_maps, core_ids=list(range(N_CORES)))
    out = np.concatenate([res.results[c]["y"].reshape(NG) for c in range(N_CORES)])
    return out.reshape(B, 1).astype(F32)



# revision 3
# speedup vs baseline: 1.3087x; 1.3087x over previous
"""Trainium2 Bass kernel for nn_CGRegressorAdapter (GNN message passing).

Strategy:
  - Data-parallel over B=32 graphs: 8 cores x 4 graphs each. Weights replicated.
  - Per-graph dense adjacency AT[src, dst] (edge-count matrix) built on host
    from edge_index (pure integer layout prep), shipped bf16 (counts are exact).
  - All node states kept transposed [128 feat, 2048 nodes] in f32.
  - GraphConv: m = h @ Wnbr via f32 PE matmuls; m split into bf16 hi+lo;
    agg^T accumulated as (m_hi^T + m_lo^T) @ AT rows streamed 512-wide (bf16 PE),
    plus f32 Wself path, all into the same PSUM; fused bias+ReLU on ACT.
  - Last-node extraction via one-hot column mask + DVE multiply-reduce.
  - Small regression head entirely on-chip in f32.
"""
import numpy as np
import ml_dtypes

import concourse.bass as bass
import concourse.mybir as mybir
from concourse import bacc
from concourse.bass import ts
from concourse.bass_utils import run_bass_kernel_spmd
from concourse.tile import TileContext

BF16 = ml_dtypes.bfloat16
F32 = np.float32

B, N, E, H, L, VOCAB = 32, 2048, 8192, 128, 4, 32
N_CORES = 8
NG = B // N_CORES          # graphs per core
NJ = N // 128              # 16 src chunks
NSPAN = N // 512           # 4 psum spans
dt = mybir.dt
Alu = mybir.AluOpType
Act = mybir.ActivationFunctionType

# bias column indices in the packed bias tile
BCOL_BASE = 0      # 0..3  base_b
BCOL_ADAPT = 4     # 4..7  adapt_b
BCOL_HB1 = 8
BCOL_HMID = 9      # 9..11
BCOL_HB5 = 12
NBCOL = 16


def _build_program(n_graphs=NG, l_base=L, l_adapt=L, do_head=True):
    nc = bacc.Bacc("TRN2", target_bir_lowering=False, debug=False,
                   num_devices=N_CORES)
    f32, bf16 = dt.float32, dt.bfloat16

    at_d = nc.declare_dram_parameter("at", [NG * NJ, 128, N], bf16, isOutput=False)
    erhs_d = nc.declare_dram_parameter("embed_rhs", [NG, 128, N], f32, isOutput=False)
    sel_d = nc.declare_dram_parameter("selrep", [NG, 128, N], bf16, isOutput=False)
    embw_d = nc.declare_dram_parameter("embed_w", [128, H], f32, isOutput=False)
    bws_d = nc.declare_dram_parameter("bwself", [L, H, H], f32, isOutput=False)
    bwn_d = nc.declare_dram_parameter("bwnbr", [L, H, H], f32, isOutput=False)
    aws_d = nc.declare_dram_parameter("awself", [L, H, 2, H], f32, isOutput=False)
    awn_d = nc.declare_dram_parameter("awnbr", [L, H, 2, H], f32, isOutput=False)
    hw1_d = nc.declare_dram_parameter("hw1", [H, 2, H], f32, isOutput=False)
    hwm_d = nc.declare_dram_parameter("hwmid", [H, 3, H], f32, isOutput=False)
    hw5_d = nc.declare_dram_parameter("hw5", [H, 1], f32, isOutput=False)
    bias_d = nc.declare_dram_parameter("biases", [H, NBCOL], f32, isOutput=False)
    y_d = nc.declare_dram_parameter("y", [1, NG], f32, isOutput=True)

    with TileContext(nc) as tc:
        with (
            tc.tile_pool(name="const", bufs=1) as const,
            tc.tile_pool(name="atp", bufs=1) as atp,
            tc.tile_pool(name="state", bufs=1) as state,
            tc.tile_pool(name="currp", bufs=2) as currp,
            tc.tile_pool(name="mp", bufs=4) as mp,
            tc.tile_pool(name="work", bufs=2) as work,
            tc.tile_pool(name="psum_agg", bufs=1, space="PSUM") as psum_agg,
            tc.tile_pool(name="psum_m", bufs=4, space="PSUM") as psum_m,
        ):
            # ---- constants ----
            embw = const.tile([128, H], f32)
            nc.sync.dma_start(embw[:], embw_d[:])
            bias_t = const.tile([H, NBCOL], f32)
            nc.sync.dma_start(bias_t[:], bias_d[:])
            bws_t = []
            bwn_t = []
            aws_t = []
            awn_t = []
            for i in range(L):
                w = const.tile([H, H], f32, tag=f"bws{i}")
                nc.sync.dma_start(w[:], bws_d[i])
                bws_t.append(w)
                w = const.tile([H, H], f32, tag=f"bwn{i}")
                nc.sync.dma_start(w[:], bwn_d[i])
                bwn_t.append(w)
                w = const.tile([H, 2, H], f32, tag=f"aws{i}")
                nc.sync.dma_start(w[:], aws_d[i])
                aws_t.append(w)
                w = const.tile([H, 2, H], f32, tag=f"awn{i}")
                nc.sync.dma_start(w[:], awn_d[i])
                awn_t.append(w)
            hw1_t = const.tile([H, 2, H], f32)
            nc.sync.dma_start(hw1_t[:], hw1_d[:])
            hwm_t = const.tile([H, 3, H], f32)
            nc.sync.dma_start(hwm_t[:], hwm_d[:])
            hw5_t = const.tile([H, 1], f32)
            nc.sync.dma_start(hw5_t[:], hw5_d[:])
            dummy = const.tile([128, 1], f32)

            gbT = state.tile([128, NG], f32, tag="gb")
            gaT = state.tile([128, NG], f32, tag="ga")

            def gconv(nbr_srcs, self_srcs, at_tiles, bias_col, out_tile,
                      relu=True):
                """nbr_srcs: list of (stateT_tile, Wnbr_rhs_ap [128,H]).
                self_srcs: list of (stateT_tile, Wself_lhsT_ap [128,H]).
                out_tile: [128, N] f32 output state."""
                aggs = [psum_agg.tile([128, 512], f32, tag=f"agg{s}",
                                      name=f"agg{s}") for s in range(NSPAN)]

                def emit_m(j):
                    pm = psum_m.tile([128, 128], f32, tag="pm")
                    nlast = len(nbr_srcs) - 1
                    for idx, (src, w) in enumerate(nbr_srcs):
                        nc.tensor.matmul(pm[:], src[:, ts(j, 128)], w,
                                         start=(idx == 0), stop=(idx == nlast))
                    mhi = mp.tile([128, 128], bf16, tag="mhi")
                    nc.scalar.copy(mhi[:], pm[:])
                    return mhi

                # first m chunks before the self path so the hi/lo casts hide
                # under the self matmuls and the A-phase starts stall-free
                m_next = emit_m(0)
                # self path: f32, wide rhs
                for idx, (src, w) in enumerate(self_srcs):
                    for s in range(NSPAN):
                        nc.tensor.matmul(aggs[s][:], w, src[:, ts(s, 512)],
                                         start=(idx == 0), stop=False)
                for j in range(NJ):
                    mhi = m_next
                    if j + 1 < NJ:
                        m_next = emit_m(j + 1)
                    for s in range(NSPAN):
                        nc.tensor.matmul(aggs[s][:], mhi[:],
                                         at_tiles[j][:, ts(s, 512)],
                                         start=False, stop=(j == NJ - 1))
                func = Act.Relu if relu else Act.Identity
                for s in range(NSPAN):
                    nc.scalar.activation(out_tile[:, ts(s, 512)],
                                         aggs[s][:], func,
                                         bias=bias_t[:, bias_col:bias_col + 1])

            for g in range(n_graphs):
                at_t = []
                for j in range(NJ):
                    t = atp.tile([128, N], bf16, tag=f"at{j}", name=f"at{j}")
                    nc.sync.dma_start(t[:], at_d[g * NJ + j])
                    at_t.append(t)
                # ---- embed ----
                erhs = work.tile([128, N], f32, tag="erhs")
                nc.sync.dma_start(erhs[:], erhs_d[g])
                selt = work.tile([128, N], bf16, tag="sel")
                nc.sync.dma_start(selt[:], sel_d[g])

                lat = [state.tile([128, N], f32, tag=f"lat{k}", name=f"lat{k}")
                       for k in range(L + 1)]
                eaggs = [psum_agg.tile([128, 512], f32, tag=f"agg{s}",
                                       name=f"eagg{s}") for s in range(NSPAN)]
                for s in range(NSPAN):
                    nc.tensor.matmul(eaggs[s][:], embw[:],
                                     erhs[:, ts(s, 512)], start=True, stop=True)
                for s in range(NSPAN):
                    nc.scalar.copy(lat[0][:, ts(s, 512)], eaggs[s][:])

                # ---- base stack ----
                for i in range(l_base):
                    gconv(nbr_srcs=[(lat[i], bwn_t[i][:])],
                          self_srcs=[(lat[i], bws_t[i][:])],
                          at_tiles=at_t, bias_col=BCOL_BASE + i,
                          out_tile=lat[i + 1])

                # ---- adapter stack ----
                curr = lat[0]
                for i in range(l_adapt):
                    ncurr = currp.tile([128, N], f32, tag="curr")
                    gconv(nbr_srcs=[(lat[i + 1], awn_t[i][:, 0, :]),
                                    (curr, awn_t[i][:, 1, :])],
                          self_srcs=[(lat[i + 1], aws_t[i][:, 0, :]),
                                     (curr, aws_t[i][:, 1, :])],
                          at_tiles=at_t, bias_col=BCOL_ADAPT + i,
                          out_tile=ncurr)
                    curr = ncurr

                # ---- last-node extraction (mask-multiply + reduce) ----
                extr = work.tile([128, N], f32, tag="extr")
                nc.vector.tensor_mul(out=extr[:], in0=lat[l_base][:], in1=selt[:])
                nc.vector.tensor_reduce(gbT[:, g:g + 1], extr[:],
                                        mybir.AxisListType.X, Alu.add)
                extr2 = work.tile([128, N], f32, tag="extr")
                nc.vector.tensor_mul(out=extr2[:], in0=curr[:], in1=selt[:])
                nc.vector.tensor_reduce(gaT[:, g:g + 1], extr2[:],
                                        mybir.AxisListType.X, Alu.add)

            if do_head:
                # ---- regression head (all graphs at once) ----
                def head_mm(lhsT, rhs, bias_col, func):
                    pm = psum_m.tile([128, 128], f32, tag="pm")
                    nc.tensor.matmul(pm[:, :NG], lhsT, rhs, start=True, stop=True)
                    out = state.tile([128, NG], f32, tag="hy")
                    nc.scalar.activation(out[:], pm[:, :NG], func,
                                         bias=bias_t[:, bias_col:bias_col + 1])
                    return out

                pm = psum_m.tile([128, 128], f32, tag="pm")
                nc.tensor.matmul(pm[:, :NG], hw1_t[:, 0, :], gbT[:], start=True, stop=False)
                nc.tensor.matmul(pm[:, :NG], hw1_t[:, 1, :], gaT[:], start=False, stop=True)
                y1 = state.tile([128, NG], f32, tag="hy")
                nc.scalar.activation(y1[:], pm[:, :NG], Act.Identity,
                                     bias=bias_t[:, BCOL_HB1:BCOL_HB1 + 1])
                y2 = head_mm(hwm_t[:, 0, :], y1[:], BCOL_HMID + 0, Act.Relu)
                y3 = head_mm(hwm_t[:, 1, :], y2[:], BCOL_HMID + 1, Act.Identity)
                y4 = head_mm(hwm_t[:, 2, :], y3[:], BCOL_HMID + 2, Act.Relu)
                pm5 = psum_m.tile([128, 128], f32, tag="pm")
                nc.tensor.matmul(pm5[:1, :NG], hw5_t[:], y4[:], start=True, stop=True)
                yout = state.tile([1, NG], f32, tag="yout")
                nc.scalar.activation(yout[:], pm5[:1, :NG], Act.Identity,
                                     bias=bias_t[:1, BCOL_HB5:BCOL_HB5 + 1])
                nc.sync.dma_start(y_d[:], yout[:])
            else:
                yout = state.tile([1, NG], f32, tag="yout")
                nc.vector.tensor_copy(out=yout[:], in_=gbT[:1, :])
                nc.sync.dma_start(y_d[:], yout[:])

    nc.compile()
    return nc


_NC_CACHE = {}


def _get_program():
    if "nc" not in _NC_CACHE:
        _NC_CACHE["nc"] = _build_program()
    return _NC_CACHE["nc"]


def _prep_inputs(inputs):
    """Host-side sharding + layout prep. Returns list of per-core in_maps."""
    inds = np.asarray(inputs["regular_node_inds"]).astype(np.int64)
    shapes = np.asarray(inputs["regular_node_shapes"], dtype=F32)
    edge = np.asarray(inputs["edge_index"]).astype(np.int64)
    last_idx = np.asarray(inputs["last_idx"]).astype(np.int64)

    # adjacency AT[src, dst] counts per graph, bf16 (exact small ints)
    at_all = np.zeros((B, N, N), dtype=F32)
    for g in range(B):
        np.add.at(at_all[g], (edge[g, 0], edge[g, 1]), 1.0)
    at_all = at_all.astype(BF16)

    # embed rhs: rows 0..31 one-hot(inds)^T, rows 32..35 shapes^T, rest 0
    erhs_all = np.zeros((B, 128, N), dtype=F32)
    ar = np.arange(N)
    for g in range(B):
        erhs_all[g, inds[g], ar] = 1.0
        erhs_all[g, VOCAB:VOCAB + 4, :] = shapes[g].T
    # last-node selection mask replicated over partitions
    sel_all = np.zeros((B, 128, N), dtype=BF16)
    for g in range(B):
        sel_all[g, :, last_idx[g]] = 1.0

    embed_w = np.zeros((128, H), dtype=F32)
    embed_w[:VOCAB] = np.asarray(inputs["embed_table"], dtype=F32)
    embed_w[VOCAB:VOCAB + 4] = np.asarray(inputs["shape_w"], dtype=F32)

    aws = np.asarray(inputs["adapt_Wself"], dtype=F32).reshape(L, 2, H, H)
    awn = np.asarray(inputs["adapt_Wnbr"], dtype=F32).reshape(L, 2, H, H)
    aws = np.ascontiguousarray(aws.transpose(0, 2, 1, 3))  # [L, H, 2, H]
    awn = np.ascontiguousarray(awn.transpose(0, 2, 1, 3))
    hw1 = np.ascontiguousarray(
        np.asarray(inputs["hW1"], dtype=F32).reshape(2, H, H).transpose(1, 0, 2))

    biases = np.zeros((H, NBCOL), dtype=F32)
    biases[:, BCOL_BASE:BCOL_BASE + L] = np.asarray(inputs["base_b"], dtype=F32).T
    biases[:, BCOL_ADAPT:BCOL_ADAPT + L] = np.asarray(inputs["adapt_b"], dtype=F32).T
    biases[:, BCOL_HB1] = np.asarray(inputs["hb1"], dtype=F32)
    biases[:, BCOL_HMID:BCOL_HMID + 3] = np.asarray(inputs["hbmid"], dtype=F32).T
    biases[0, BCOL_HB5] = np.asarray(inputs["hb5"], dtype=F32)[0]

    shared = {
        "embed_w": embed_w,
        "bwself": np.asarray(inputs["base_Wself"], dtype=F32),
        "bwnbr": np.asarray(inputs["base_Wnbr"], dtype=F32),
        "awself": aws,
        "awnbr": awn,
        "hw1": hw1,
        "hwmid": np.ascontiguousarray(
            np.asarray(inputs["hWmid"], dtype=F32).transpose(1, 0, 2)),
        "hw5": np.asarray(inputs["hW5"], dtype=F32),
        "biases": biases,
    }
    in_maps = []
    for c in range(N_CORES):
        g0 = c * NG
        in_maps.append({
            "at": np.ascontiguousarray(
                at_all[g0:g0 + NG].reshape(NG * NJ, 128, N)),
            "embed_rhs": erhs_all[g0:g0 + NG],
            "selrep": sel_all[g0:g0 + NG],
            **shared,
        })
    return in_maps


def kernel(**inputs) -> np.ndarray:
    nc = _get_program()
    in_maps = _prep_inputs(inputs)
    res = run_bass_kernel_spmd(nc, in_maps, core_ids=list(range(N_CORES)))
    out = np.concatenate([res.results[c]["y"].reshape(NG) for c in range(N_CORES)])
    return out.reshape(B, 1).astype(F32)



# revision 12
# speedup vs baseline: 942910.0000x; 720481.0000x over previous
"""Trainium2 Bass kernel for nn_CGRegressorAdapter (GNN message passing).

Strategy:
  - Data-parallel over B=32 graphs: 8 cores x 4 graphs each. Weights replicated.
  - Per-graph dense adjacency AT[src, dst] (edge-count matrix) built on host
    from edge_index (pure integer layout prep), shipped bf16 (counts are exact).
  - All node states kept transposed [128 feat, 2048 nodes] in f32.
  - GraphConv: m = h @ Wnbr via f32 PE matmuls; m split into bf16 hi+lo;
    agg^T accumulated as (m_hi^T + m_lo^T) @ AT rows streamed 512-wide (bf16 PE),
    plus f32 Wself path, all into the same PSUM; fused bias+ReLU on ACT.
  - Last-node extraction via one-hot column mask + DVE multiply-reduce.
  - Small regression head entirely on-chip in f32.
"""
import numpy as np
import ml_dtypes

import concourse.bass as bass
import concourse.mybir as mybir
from concourse import bacc
from concourse.bass import ts
from concourse.bass_utils import run_bass_kernel_spmd
from concourse.tile import TileContext

BF16 = ml_dtypes.bfloat16
FP8 = ml_dtypes.float8_e4m3
F32 = np.float32

B, N, E, H, L, VOCAB = 32, 2048, 8192, 128, 4, 32
N_CORES = 8
NG = B // N_CORES          # graphs per core
NJ = N // 128              # 16 src chunks
NSPAN = N // 512           # 4 psum spans
dt = mybir.dt
Alu = mybir.AluOpType
Act = mybir.ActivationFunctionType

# bias column indices in the packed bias tile
BCOL_BASE = 0      # 0..3  base_b
BCOL_ADAPT = 4     # 4..7  adapt_b
BCOL_HB1 = 8
BCOL_HMID = 9      # 9..11
BCOL_HB5 = 12
NBCOL = 16


def _build_program(n_graphs=NG, l_base=L, l_adapt=L, do_head=True):
    nc = bacc.Bacc("TRN2", target_bir_lowering=False, debug=False,
                   num_devices=N_CORES)
    f32, bf16, fp8 = dt.float32, dt.bfloat16, dt.float8e4

    at_d = nc.declare_dram_parameter("at", [NG * NJ, 128, N], fp8, isOutput=False)
    erhs_d = nc.declare_dram_parameter("embed_rhs", [NG, 128, N], f32, isOutput=False)
    sel_d = nc.declare_dram_parameter("selrep", [NG, 128, N], bf16, isOutput=False)
    embw_d = nc.declare_dram_parameter("embed_w", [128, H], f32, isOutput=False)
    bws_d = nc.declare_dram_parameter("bwself", [L, H, H], f32, isOutput=False)
    bwn_d = nc.declare_dram_parameter("bwnbr", [L, H, H], f32, isOutput=False)
    aws_d = nc.declare_dram_parameter("awself", [L, H, 2, H], f32, isOutput=False)
    awn_d = nc.declare_dram_parameter("awnbr", [L, H, 2, H], f32, isOutput=False)
    hw1_d = nc.declare_dram_parameter("hw1", [H, 2, H], f32, isOutput=False)
    hwm_d = nc.declare_dram_parameter("hwmid", [H, 3, H], f32, isOutput=False)
    hw5_d = nc.declare_dram_parameter("hw5", [H, 1], f32, isOutput=False)
    bias_d = nc.declare_dram_parameter("biases", [H, NBCOL], f32, isOutput=False)
    y_d = nc.declare_dram_parameter("y", [1, NG], f32, isOutput=True)

    with TileContext(nc) as tc:
        with (
            tc.tile_pool(name="const", bufs=1) as const,
            tc.tile_pool(name="atp", bufs=2) as atp,
            tc.tile_pool(name="state", bufs=1) as state,
            tc.tile_pool(name="currp", bufs=2) as currp,
            tc.tile_pool(name="mp", bufs=4) as mp,
            tc.tile_pool(name="work", bufs=2) as work,
            tc.tile_pool(name="psum_agg", bufs=1, space="PSUM") as psum_agg,
            tc.tile_pool(name="psum_m", bufs=4, space="PSUM") as psum_m,
        ):
            # ---- constants ----
            embw = const.tile([128, H], f32)
            nc.sync.dma_start(embw[:], embw_d[:])
            bias_t = const.tile([H, NBCOL], f32)
            nc.sync.dma_start(bias_t[:], bias_d[:])
            bws_t = []
            bwn_t = []
            aws_t = []
            awn_t = []
            for i in range(L):
                w = const.tile([H, H], f32, tag=f"bws{i}")
                nc.sync.dma_start(w[:], bws_d[i])
                bws_t.append(w)
                w = const.tile([H, H], f32, tag=f"bwn{i}")
                nc.sync.dma_start(w[:], bwn_d[i])
                bwn_t.append(w)
                w = const.tile([H, 2, H], f32, tag=f"aws{i}")
                nc.sync.dma_start(w[:], aws_d[i])
                aws_t.append(w)
                w = const.tile([H, 2, H], f32, tag=f"awn{i}")
                nc.sync.dma_start(w[:], awn_d[i])
                awn_t.append(w)
            hw1_t = const.tile([H, 2, H], f32)
            nc.sync.dma_start(hw1_t[:], hw1_d[:])
            hwm_t = const.tile([H, 3, H], f32)
            nc.sync.dma_start(hwm_t[:], hwm_d[:])
            hw5_t = const.tile([H, 1], f32)
            nc.sync.dma_start(hw5_t[:], hw5_d[:])
            dummy = const.tile([128, 1], f32)

            gbT = state.tile([128, NG], f32, tag="gb")
            gaT = state.tile([128, NG], f32, tag="ga")

            def gconv(nbr_srcs, self_srcs, at_tiles, bias_col, out_tile,
                      relu=True):
                """nbr_srcs: list of (stateT_tile, Wnbr_rhs_ap [128,H]).
                self_srcs: list of (stateT_tile, Wself_lhsT_ap [128,H]).
                out_tile: [128, N] f32 output state."""
                aggs = [psum_agg.tile([128, 512], f32, tag=f"agg{s}",
                                      name=f"agg{s}") for s in range(NSPAN)]

                def emit_m(j):
                    pm = psum_m.tile([128, 128], f32, tag="pm")
                    nlast = len(nbr_srcs) - 1
                    for idx, (src, w) in enumerate(nbr_srcs):
                        nc.tensor.matmul(pm[:], src[:, ts(j, 128)], w,
                                         start=(idx == 0), stop=(idx == nlast))
                    mhi = mp.tile([128, 128], bf16, tag="mhi")
                    nc.scalar.copy(mhi[:], pm[:])
                    return mhi

                m_next = emit_m(0)
                for idx, (src, w) in enumerate(self_srcs):
                    for s in range(NSPAN):
                        nc.tensor.matmul(aggs[s][:], w, src[:, ts(s, 512)],
                                         start=(idx == 0), stop=False)
                for j in range(NJ):
                    mhi = m_next
                    if j + 1 < NJ:
                        m_next = emit_m(j + 1)
                    for s in range(NSPAN):
                        nc.tensor.matmul(aggs[s][:], mhi[:],
                                         at_tiles[j][:, ts(s, 512)],
                                         start=False, stop=(j == NJ - 1))
                func = Act.Relu if relu else Act.Identity
                for s in range(NSPAN):
                    nc.scalar.activation(out_tile[:, ts(s, 512)],
                                         aggs[s][:], func,
                                         bias=bias_t[:, bias_col:bias_col + 1])

            for g in range(n_graphs):
                # ---- embed + mask DMAs first (critical path for layer 0) ----
                erhs = work.tile([128, N], f32, tag="erhs")
                nc.sync.dma_start(erhs[:], erhs_d[g])
                selt = work.tile([128, N], bf16, tag="sel")
                nc.sync.dma_start(selt[:], sel_d[g])
                at_t = []
                for j in range(NJ):
                    t = atp.tile([128, N], fp8, tag=f"at{j}", name=f"at{j}")
                    nc.sync.dma_start(t[:], at_d[g * NJ + j])
                    at_t.append(t)

                lat = [state.tile([128, N], f32, tag=f"lat{k}", name=f"lat{k}")
                       for k in range(L + 1)]
                eaggs = [psum_agg.tile([128, 512], f32, tag=f"agg{s}",
                                       name=f"eagg{s}") for s in range(NSPAN)]
                for s in range(NSPAN):
                    nc.tensor.matmul(eaggs[s][:], embw[:],
                                     erhs[:, ts(s, 512)], start=True, stop=True)
                for s in range(NSPAN):
                    nc.scalar.copy(lat[0][:, ts(s, 512)], eaggs[s][:])

                # ---- base stack ----
                for i in range(l_base):
                    gconv(nbr_srcs=[(lat[i], bwn_t[i][:])],
                          self_srcs=[(lat[i], bws_t[i][:])],
                          at_tiles=at_t, bias_col=BCOL_BASE + i,
                          out_tile=lat[i + 1])

                # ---- adapter stack ----
                curr = lat[0]
                for i in range(l_adapt):
                    ncurr = currp.tile([128, N], f32, tag="curr")
                    gconv(nbr_srcs=[(lat[i + 1], awn_t[i][:, 0, :]),
                                    (curr, awn_t[i][:, 1, :])],
                          self_srcs=[(lat[i + 1], aws_t[i][:, 0, :]),
                                     (curr, aws_t[i][:, 1, :])],
                          at_tiles=at_t, bias_col=BCOL_ADAPT + i,
                          out_tile=ncurr)
                    curr = ncurr

                # ---- last-node extraction (mask-multiply + reduce) ----
                extr = work.tile([128, N], f32, tag="extr")
                nc.vector.tensor_mul(out=extr[:], in0=lat[l_base][:], in1=selt[:])
                nc.vector.tensor_reduce(gbT[:, g:g + 1], extr[:],
                                        mybir.AxisListType.X, Alu.add)
                extr2 = work.tile([128, N], f32, tag="extr")
                nc.vector.tensor_mul(out=extr2[:], in0=curr[:], in1=selt[:])
                nc.vector.tensor_reduce(gaT[:, g:g + 1], extr2[:],
                                        mybir.AxisListType.X, Alu.add)

            if do_head:
                # ---- regression head (all graphs at once) ----
                def head_mm(lhsT, rhs, bias_col, func):
                    pm = psum_m.tile([128, 128], f32, tag="pm")
                    nc.tensor.matmul(pm[:, :NG], lhsT, rhs, start=True, stop=True)
                    out = state.tile([128, NG], f32, tag="hy")
                    nc.scalar.activation(out[:], pm[:, :NG], func,
                                         bias=bias_t[:, bias_col:bias_col + 1])
                    return out

                pm = psum_m.tile([128, 128], f32, tag="pm")
                nc.tensor.matmul(pm[:, :NG], hw1_t[:, 0, :], gbT[:], start=True, stop=False)
                nc.tensor.matmul(pm[:, :NG], hw1_t[:, 1, :], gaT[:], start=False, stop=True)
                y1 = state.tile([128, NG], f32, tag="hy")
                nc.scalar.activation(y1[:], pm[:, :NG], Act.Identity,
                                     bias=bias_t[:, BCOL_HB1:BCOL_HB1 + 1])
                y2 = head_mm(hwm_t[:, 0, :], y1[:], BCOL_HMID + 0, Act.Relu)
                y3 = head_mm(hwm_t[:, 1, :], y2[:], BCOL_HMID + 1, Act.Identity)
                y4 = head_mm(hwm_t[:, 2, :], y3[:], BCOL_HMID + 2, Act.Relu)
                pm5 = psum_m.tile([128, 128], f32, tag="pm")
                nc.tensor.matmul(pm5[:1, :NG], hw5_t[:], y4[:], start=True, stop=True)
                yout = state.tile([1, NG], f32, tag="yout")
                nc.scalar.activation(yout[:], pm5[:1, :NG], Act.Identity,
                                     bias=bias_t[:1, BCOL_HB5:BCOL_HB5 + 1])
                nc.sync.dma_start(y_d[:], yout[:])
            else:
                yout = state.tile([1, NG], f32, tag="yout")
                nc.vector.tensor_copy(out=yout[:], in_=gbT[:1, :])
                nc.sync.dma_start(y_d[:], yout[:])

    nc.compile()
    return nc


_NC_CACHE = {}


def _get_program():
    if "nc" not in _NC_CACHE:
        _NC_CACHE["nc"] = _build_program()
    return _NC_CACHE["nc"]


def _prep_inputs(inputs):
    """Host-side sharding + layout prep. Returns list of per-core in_maps."""
    inds = np.asarray(inputs["regular_node_inds"]).astype(np.int64)
    shapes = np.asarray(inputs["regular_node_shapes"], dtype=F32)
    edge = np.asarray(inputs["edge_index"]).astype(np.int64)
    last_idx = np.asarray(inputs["last_idx"]).astype(np.int64)

    # adjacency AT[src, dst] counts per graph, fp8 e4m3 (exact small ints)
    at_all = np.zeros((B, N, N), dtype=F32)
    for g in range(B):
        np.add.at(at_all[g], (edge[g, 0], edge[g, 1]), 1.0)
    at_all = at_all.astype(FP8)

    # embed rhs: rows 0..31 one-hot(inds)^T, rows 32..35 shapes^T, rest 0
    erhs_all = np.zeros((B, 128, N), dtype=F32)
    ar = np.arange(N)
    for g in range(B):
        erhs_all[g, inds[g], ar] = 1.0
        erhs_all[g, VOCAB:VOCAB + 4, :] = shapes[g].T
    # last-node selection mask replicated over partitions
    sel_all = np.zeros((B, 128, N), dtype=BF16)
    for g in range(B):
        sel_all[g, :, last_idx[g]] = 1.0

    embed_w = np.zeros((128, H), dtype=F32)
    embed_w[:VOCAB] = np.asarray(inputs["embed_table"], dtype=F32)
    embed_w[VOCAB:VOCAB + 4] = np.asarray(inputs["shape_w"], dtype=F32)

    aws = np.asarray(inputs["adapt_Wself"], dtype=F32).reshape(L, 2, H, H)
    awn = np.asarray(inputs["adapt_Wnbr"], dtype=F32).reshape(L, 2, H, H)
    aws = np.ascontiguousarray(aws.transpose(0, 2, 1, 3))  # [L, H, 2, H]
    awn = np.ascontiguousarray(awn.transpose(0, 2, 1, 3))
    hw1 = np.ascontiguousarray(
        np.asarray(inputs["hW1"], dtype=F32).reshape(2, H, H).transpose(1, 0, 2))

    biases = np.zeros((H, NBCOL), dtype=F32)
    biases[:, BCOL_BASE:BCOL_BASE + L] = np.asarray(inputs["base_b"], dtype=F32).T
    biases[:, BCOL_ADAPT:BCOL_ADAPT + L] = np.asarray(inputs["adapt_b"], dtype=F32).T
    biases[:, BCOL_HB1] = np.asarray(inputs["hb1"], dtype=F32)
    biases[:, BCOL_HMID:BCOL_HMID + 3] = np.asarray(inputs["hbmid"], dtype=F32).T
    biases[0, BCOL_HB5] = np.asarray(inputs["hb5"], dtype=F32)[0]

    shared = {
        "embed_w": embed_w,
        "bwself": np.asarray(inputs["base_Wself"], dtype=F32),
        "bwnbr": np.asarray(inputs["base_Wnbr"], dtype=F32),
        "awself": aws,
        "awnbr": awn,
        "hw1": hw1,
        "hwmid": np.ascontiguousarray(
            np.asarray(inputs["hWmid"], dtype=F32).transpose(1, 0, 2)),
        "hw5": np.asarray(inputs["hW5"], dtype=F32),
        "biases": biases,
    }
    in_maps = []
    for c in range(N_CORES):
        g0 = c * NG
        in_maps.append({
            "at": np.ascontiguousarray(
                at_all[g0:g0 + NG].reshape(NG * NJ, 128, N)),
            "embed_rhs": erhs_all[g0:g0 + NG],
            "selrep": sel_all[g0:g0 + NG],
            **shared,
        })
    return in_maps


def kernel(**inputs) -> np.ndarray:
    nc = _get_program()
    in_maps = _prep_inputs(inputs)
    res = run_bass_kernel_spmd(nc, in_maps, core_ids=list(range(N_CORES)))
    out = np.concatenate([res.results[c]["y"].reshape(NG) for c in range(N_CORES)])
    return out.reshape(B, 1).astype(F32)

